# revision 1
# baseline (speedup 1.0000x reference)
"""Trainium2 Bass kernel for nn_Llama3_68135361184133.

Strategy: pure data-parallel over the 112 (b,m) sequences -> 14 seqs (896
tokens) per core, 8 cores, no collectives.  Compute dtype fp16 (weights +
matmul activations), fp32 accumulation, fp32 residual staged through DRAM.
Activations are feature-major ([128 chan, chunk, token]) so every matmul
contracts over the partition axis.  Per-token RMS scales are applied as
post-matmul multiplies (linearity) instead of materializing normalized
copies.  Host side: RevIN stats + patch extraction, weight swizzle/cast,
output un-scaling.
"""

import math
from contextlib import ExitStack

import numpy as np

import concourse.bass as bass
from concourse import bacc
import concourse.mybir as mybir
import concourse.tile as tile
from concourse import bass_utils
from concourse.masks import make_identity

F16 = mybir.dt.float16
F32 = mybir.dt.float32
AL = mybir.AluOpType
AF = mybir.ActivationFunctionType

B, L, M = 16, 512, 7
P_PATCH, STRIDE = 16, 8
N = 64
D, H, KV, HD, DFF = 4096, 32, 8, 128, 14336
FC, PRED = 128, 96
THETA = 500000.0
EPS = 1e-5

NCORES = 8
SEQ = 14
T = SEQ * N                # 896
C = D // 128               # 32
JC = DFF // 128            # 112
TB = 448
NTB = T // TB              # 2
JG = 14
NG = JC // JG              # 8
SPT = TB // N              # 7 sequences per token-block


def build_nc():
    nc = bacc.Bacc("TRN2")

    d = {}
    d["patches"] = nc.dram_tensor("patches", [128, T], F16, kind="ExternalInput")
    d["w_in"] = nc.dram_tensor("w_in", [128, C, 128], F16, kind="ExternalInput")
    d["b_in"] = nc.dram_tensor("b_in", [128, C], F32, kind="ExternalInput")
    d["wq"] = nc.dram_tensor("wq", [H, 128, C, 128], F16, kind="ExternalInput")
    d["wk"] = nc.dram_tensor("wk", [KV, 128, C, 128], F16, kind="ExternalInput")
    d["wv"] = nc.dram_tensor("wv", [KV, 128, C, 128], F16, kind="ExternalInput")
    d["wo"] = nc.dram_tensor("wo", [C, 128, C, 128], F16, kind="ExternalInput")
    d["wg"] = nc.dram_tensor("wg", [JC, 128, C, 128], F16, kind="ExternalInput")
    d["wu"] = nc.dram_tensor("wu", [JC, 128, C, 128], F16, kind="ExternalInput")
    d["wd"] = nc.dram_tensor("wd", [NG, C, 128, JG, 128], F16, kind="ExternalInput")
    d["wfc"] = nc.dram_tensor("wfc", [N, 128, C, FC], F16, kind="ExternalInput")
    d["wout"] = nc.dram_tensor("wout", [FC, PRED], F16, kind="ExternalInput")
    d["b_fc"] = nc.dram_tensor("b_fc", [1, FC], F32, kind="ExternalInput")
    d["b_out"] = nc.dram_tensor("b_out", [PRED, 1], F32, kind="ExternalInput")
    d["cos_q"] = nc.dram_tensor("cos_q", [128, N], F32, kind="ExternalInput")
    d["sin_q"] = nc.dram_tensor("sin_q", [128, N], F32, kind="ExternalInput")
    d["cos_k"] = nc.dram_tensor("cos_k", [128, N], F32, kind="ExternalInput")
    d["sin_k"] = nc.dram_tensor("sin_k", [128, N], F32, kind="ExternalInput")
    d["mask"] = nc.dram_tensor("mask", [N, N], F16, kind="ExternalInput")
    out_d = nc.dram_tensor("out", [PRED, SEQ], F32, kind="ExternalOutput")

    def tb(th):
        return slice(th * TB, (th + 1) * TB)

    with tile.TileContext(nc) as tc, ExitStack() as ctx:
        consts = ctx.enter_context(tc.tile_pool(name="consts", bufs=1))
        dram = ctx.enter_context(tc.tile_pool(name="dram", bufs=1, space="DRAM"))
        srep_p = ctx.enter_context(tc.tile_pool(name="srep", bufs=1))
        ssv_p = ctx.enter_context(tc.tile_pool(name="ssv", bufs=1))
        big = ctx.enter_context(tc.tile_pool(name="big", bufs=1))

        h1_d = dram.tile([C, 128, T], F32)
        s_bounce = dram.tile([1, T], F32)
        h2_d = dram.tile([C, 128, T], F32)
        h3_d = dram.tile([C, 128, T], F32)

        patches_sb = consts.tile([128, T], F16)
        nc.sync.dma_start(patches_sb[:], d["patches"][:])
        b_in_sb = consts.tile([128, C], F32)
        nc.sync.dma_start(b_in_sb[:], d["b_in"][:])
        cosq = consts.tile([128, N], F32)
        nc.sync.dma_start(cosq[:], d["cos_q"][:])
        sinq = consts.tile([128, N], F32)
        nc.sync.dma_start(sinq[:], d["sin_q"][:])
        cosk = consts.tile([128, N], F32)
        nc.sync.dma_start(cosk[:], d["cos_k"][:])
        sink = consts.tile([128, N], F32)
        nc.sync.dma_start(sink[:], d["sin_k"][:])
        mask_sb = consts.tile([N, N], F16)
        nc.sync.dma_start(mask_sb[:], d["mask"][:])
        ones128 = consts.tile([128, 1], F16)
        nc.vector.memset(ones128[:], 1.0)
        ones1 = consts.tile([1, 128], F16)
        nc.vector.memset(ones1[:], 1.0)
        ident = consts.tile([128, 128], F16)
        make_identity(nc, ident[:])
        wout_sb = consts.tile([FC, PRED], F16)
        nc.sync.dma_start(wout_sb[:], d["wout"][:])
        bfc_sb = consts.tile([SEQ, FC], F32)
        nc.sync.dma_start(bfc_sb[:], d["b_fc"][:].to_broadcast((SEQ, FC)))
        bout_sb = consts.tile([PRED, 1], F32)
        nc.sync.dma_start(bout_sb[:], d["b_out"][:])

        ss_sb = ssv_p.tile([1, T], F32, tag="ss")
        svec = ssv_p.tile([1, T], F32, tag="svec")
        s_rep = srep_p.tile([128, T], F32, tag="srep")
        eps_sb = consts.tile([1, 1], F32)
        nc.vector.memset(eps_sb[:], EPS)

        def rms_finish():
            nc.scalar.activation(svec[:], ss_sb[:], AF.Sqrt, bias=eps_sb[:],
                                 scale=1.0 / D)
            nc.vector.reciprocal(svec[:], svec[:])
            nc.sync.dma_start(s_bounce[:], svec[:])
            nc.sync.dma_start(s_rep[:], s_bounce[:].to_broadcast((128, T)))

        # slot X: h1f16 -> o_T -> delta -> h3n ; slot Y: q_T -> h2f16
        h1f16 = big.tile([128, C, T], F16, tag="X", name="h1f16")

        # ---------------- P1: patch embed ----------------
        with ExitStack() as s1:
            wsl = s1.enter_context(tc.tile_pool(name="wsl1", bufs=2))
            st = s1.enter_context(tc.tile_pool(name="st1", bufs=3))
            ps = s1.enter_context(tc.tile_pool(name="ps1", bufs=2, space="PSUM"))
            w_in_sb = wsl.tile([128, C, 128], F16, tag="win")
            nc.sync.dma_start(w_in_sb[:], d["w_in"][:])
            nc.vector.memset(ss_sb[:], 0.0)
            for c in range(C):
                for th in range(NTB):
                    pse = ps.tile([128, TB], F32, tag="mm")
                    nc.tensor.matmul(pse[:], w_in_sb[:, c, :], patches_sb[:, tb(th)],
                                     start=True, stop=True)
                    h1t = st.tile([128, TB], F32, tag="h1t")
                    nc.vector.tensor_tensor(
                        h1t[:], pse[:],
                        b_in_sb[:, c:c + 1].to_broadcast((128, TB)), AL.add)
                    nc.sync.dma_start(h1_d[c, :, tb(th)], h1t[:])
                    nc.vector.tensor_copy(h1f16[:, c, tb(th)], h1t[:])
                    sqt = st.tile([128, TB], F16, tag="sqt")
                    nc.vector.tensor_tensor(sqt[:], h1t[:], h1t[:], AL.mult)
                    pss = ps.tile([1, TB], F32, tag="ssp")
                    nc.tensor.matmul(pss[:], ones128[:], sqt[:], start=True, stop=True)
                    nc.vector.tensor_add(ss_sb[:, tb(th)], ss_sb[:, tb(th)], pss[:])
        rms_finish()  # s1

        # ---------------- P2 + P3 ----------------
        q_T = big.tile([128, H, T], F16, tag="Y", name="q_T")
        with ExitStack() as s23:
            kv = s23.enter_context(tc.tile_pool(name="kv", bufs=1))
            k_T = kv.tile([128, KV, T], F16, tag="k")
            v_N = kv.tile([64, SEQ, KV, HD], F16, tag="v")

            with ExitStack() as s2:
                wsl = s2.enter_context(tc.tile_pool(name="wsl2", bufs=3))
                rt = s2.enter_context(tc.tile_pool(name="rt2", bufs=2))
                ps = s2.enter_context(tc.tile_pool(name="ps2", bufs=2, space="PSUM"))
                pst = s2.enter_context(tc.tile_pool(name="pst2", bufs=2, space="PSUM"))

                def proj_rope(src, nf, dst, cos_t, sin_t):
                    for f in range(nf):
                        slab = wsl.tile([128, C, 128], F16, tag="w")
                        nc.sync.dma_start(slab[:], src[f])
                        for th in range(NTB):
                            p = ps.tile([128, TB], F32, tag="mm")
                            for c in range(C):
                                nc.tensor.matmul(p[:], slab[:, c, :],
                                                 h1f16[:, c, tb(th)],
                                                 start=(c == 0), stop=(c == C - 1))
                            pv = p[:].rearrange("p (s n) -> p s n", n=N)
                            t1 = rt.tile([128, TB], F32, tag="t1")
                            t2 = rt.tile([128, TB], F32, tag="t2")
                            t1v = t1[:].rearrange("p (s n) -> p s n", n=N)
                            t2v = t2[:].rearrange("p (s n) -> p s n", n=N)
                            cb = cos_t[:, None, :].to_broadcast((128, SPT, N))
                            nc.vector.tensor_tensor(t1v, pv, cb, AL.mult)
                            slo = sin_t[0:64][:, None, :].to_broadcast((64, SPT, N))
                            shi = sin_t[64:128][:, None, :].to_broadcast((64, SPT, N))
                            nc.vector.tensor_tensor(t2v[0:64], pv[64:128], slo, AL.mult)
                            nc.vector.tensor_tensor(t2v[64:128], pv[0:64], shi, AL.mult)
                            nc.vector.tensor_add(t1[:], t1[:], t2[:])
                            nc.vector.tensor_tensor(dst[:, f, tb(th)], t1[:],
                                                    s_rep[:, tb(th)], AL.mult)

                proj_rope(d["wq"], H, q_T, cosq, sinq)
                proj_rope(d["wk"], KV, k_T, cosk, sink)

                for f in range(KV):
                    slab = wsl.tile([128, C, 128], F16, tag="w")
                    nc.sync.dma_start(slab[:], d["wv"][f])
                    for th in range(NTB):
                        p = ps.tile([128, TB], F32, tag="mm")
                        for c in range(C):
                            nc.tensor.matmul(p[:], slab[:, c, :], h1f16[:, c, tb(th)],
                                             start=(c == 0), stop=(c == C - 1))
                        vt = rt.tile([128, TB], F16, tag="vt")
                        nc.vector.tensor_tensor(vt[:], p[:], s_rep[:, tb(th)], AL.mult)
                        for si in range(SPT):
                            s = th * SPT + si
                            ptr = pst.tile([N, 128], F16, tag="tr")
                            nc.tensor.transpose(ptr[:], vt[:, si * N:(si + 1) * N],
                                                ident[:])
                            nc.vector.tensor_copy(v_N[:, s, f, :], ptr[:])

            # P3: attention (slot X reused: o_T)
            o_T = big.tile([128, C, T], F16, tag="X", name="o_T")
            with ExitStack() as s3:
                at = s3.enter_context(tc.tile_pool(name="at3", bufs=3))
                ps = s3.enter_context(tc.tile_pool(name="ps3", bufs=2, space="PSUM"))
                for s in range(SEQ):
                    sl = slice(s * N, (s + 1) * N)
                    for g in range(KV):
                        qs = q_T[:, 4 * g:4 * g + 4, sl]
                        ks = k_T[:, g, sl]
                        psp = ps.tile([N, 4, N], F32, tag="pp")
                        nc.tensor.matmul(psp[:], ks, qs, start=True, stop=True)
                        pt_sb = at.tile([N, 4, N], F16, tag="pt")
                        nc.scalar.activation(pt_sb[:], psp[:], AF.Exp)
                        mb = mask_sb[:, None, :].to_broadcast((N, 4, N))
                        nc.vector.tensor_tensor(pt_sb[:], pt_sb[:], mb, AL.mult)
                        ptf = pt_sb[:].rearrange("k h n -> k (h n)")
                        psd = ps.tile([1, 4 * N], F32, tag="pd")
                        nc.tensor.matmul(psd[:], ones128[0:N, :], ptf,
                                         start=True, stop=True)
                        rv = at.tile([1, 4 * N], F16, tag="rv")
                        with nc.allow_low_precision(reason="softmax denom in f16"):
                            nc.vector.reciprocal(rv[:], psd[:])
                        psb = ps.tile([128, 4 * N], F32, tag="pb")
                        nc.tensor.matmul(psb[:], ones1[:], rv[:], start=True, stop=True)
                        r2 = at.tile([128, 4 * N], F16, tag="r2")
                        nc.scalar.copy(r2[:], psb[:])
                        vs = v_N[:, s, g, :]
                        pso = ps.tile([128, 4 * N], F32, tag="po")
                        nc.tensor.matmul(pso[:], vs, ptf, start=True, stop=True)
                        ov = o_T[:, 4 * g:4 * g + 4, sl]
                        pso3 = pso[:].rearrange("p (h n) -> p h n", n=N)
                        r23 = r2[:].rearrange("p (h n) -> p h n", n=N)
                        nc.vector.tensor_tensor(ov, pso3, r23, AL.mult)

        # ---------------- P4: Wo + residual ----------------
        h2f16 = big.tile([128, C, T], F16, tag="Y", name="h2f16")
        with ExitStack() as s4:
            wsl = s4.enter_context(tc.tile_pool(name="wsl4", bufs=2))
            st = s4.enter_context(tc.tile_pool(name="st4", bufs=3))
            ps = s4.enter_context(tc.tile_pool(name="ps4", bufs=2, space="PSUM"))
            nc.vector.memset(ss_sb[:], 0.0)
            for f in range(C):
                slab = wsl.tile([128, C, 128], F16, tag="w")
                nc.sync.dma_start(slab[:], d["wo"][f])
                for th in range(NTB):
                    p = ps.tile([128, TB], F32, tag="mm")
                    for c in range(C):
                        nc.tensor.matmul(p[:], slab[:, c, :], o_T[:, c, tb(th)],
                                         start=(c == 0), stop=(c == C - 1))
                    h1t = st.tile([128, TB], F32, tag="h1t")
                    nc.sync.dma_start(h1t[:], h1_d[f, :, tb(th)])
                    h2t = st.tile([128, TB], F32, tag="h2t")
                    nc.vector.tensor_add(h2t[:], p[:], h1t[:])
                    nc.sync.dma_start(h2_d[f, :, tb(th)], h2t[:])
                    nc.vector.tensor_copy(h2f16[:, f, tb(th)], h2t[:])
                    sqt = st.tile([128, TB], F16, tag="sqt")
                    nc.vector.tensor_tensor(sqt[:], h2t[:], h2t[:], AL.mult)
                    pss = ps.tile([1, TB], F32, tag="ssp")
                    nc.tensor.matmul(pss[:], ones128[:], sqt[:], start=True, stop=True)
                    nc.vector.tensor_add(ss_sb[:, tb(th)], ss_sb[:, tb(th)], pss[:])
        rms_finish()  # s2

        # ---------------- P5: SwiGLU MLP ----------------
        delta = big.tile([128, C, TB], F32, tag="X", name="delta")
        with ExitStack() as s5:
            wsl = s5.enter_context(tc.tile_pool(name="wsl5", bufs=3))
            agp = s5.enter_context(tc.tile_pool(name="ag5", bufs=2))
            mt = s5.enter_context(tc.tile_pool(name="mt5", bufs=2))
            st = s5.enter_context(tc.tile_pool(name="st5", bufs=2))
            ps = s5.enter_context(tc.tile_pool(name="ps5", bufs=2, space="PSUM"))
            nc.vector.memset(ss_sb[:], 0.0)
            for th in range(NTB):
                for g in range(NG):
                    a_g = agp.tile([128, JG, TB], F16, tag="ag")
                    for jj in range(JG):
                        j = g * JG + jj
                        gs = wsl.tile([128, C, 128], F16, tag="wgu")
                        nc.sync.dma_start(gs[:], d["wg"][j])
                        us = wsl.tile([128, C, 128], F16, tag="wgu")
                        nc.sync.dma_start(us[:], d["wu"][j])
                        psg = ps.tile([128, TB], F32, tag="g")
                        psu = ps.tile([128, TB], F32, tag="u")
                        for c in range(C):
                            nc.tensor.matmul(psg[:], gs[:, c, :], h2f16[:, c, tb(th)],
                                             start=(c == 0), stop=(c == C - 1))
                        for c in range(C):
                            nc.tensor.matmul(psu[:], us[:, c, :], h2f16[:, c, tb(th)],
                                             start=(c == 0), stop=(c == C - 1))
                        t1 = mt.tile([128, TB], F16, tag="gt")
                        nc.vector.tensor_tensor(t1[:], psg[:], s_rep[:, tb(th)], AL.mult)
                        sg = mt.tile([128, TB], F16, tag="sg")
                        nc.scalar.activation(sg[:], t1[:], AF.Silu)
                        nc.vector.tensor_tensor(a_g[:, jj, :], sg[:], psu[:], AL.mult)
                    for f in range(C):
                        dsl = wsl.tile([128, JG, 128], F16, tag="wd")
                        nc.sync.dma_start(dsl[:], d["wd"][g, f])
                        psd = ps.tile([128, TB], F32, tag="dd")
                        for jj in range(JG):
                            nc.tensor.matmul(psd[:], dsl[:, jj, :], a_g[:, jj, :],
                                             start=(jj == 0), stop=(jj == JG - 1))
                        if g == 0:
                            nc.vector.tensor_copy(delta[:, f, :], psd[:])
                        else:
                            nc.vector.tensor_add(delta[:, f, :], delta[:, f, :], psd[:])
                for f in range(C):
                    h2t = st.tile([128, TB], F32, tag="h2t")
                    nc.sync.dma_start(h2t[:], h2_d[f, :, tb(th)])
                    dt = st.tile([128, TB], F32, tag="dt")
                    nc.vector.tensor_tensor(dt[:], delta[:, f, :], s_rep[:, tb(th)],
                                            AL.mult)
                    h3t = st.tile([128, TB], F32, tag="h3t")
                    nc.vector.tensor_add(h3t[:], h2t[:], dt[:])
                    nc.sync.dma_start(h3_d[f, :, tb(th)], h3t[:])
                    sqt = st.tile([128, TB], F16, tag="sqt")
                    nc.vector.tensor_tensor(sqt[:], h3t[:], h3t[:], AL.mult)
                    pss = ps.tile([1, TB], F32, tag="ssp")
                    nc.tensor.matmul(pss[:], ones128[:], sqt[:], start=True, stop=True)
                    nc.vector.tensor_add(ss_sb[:, tb(th)], ss_sb[:, tb(th)], pss[:])
        rms_finish()  # s3

        # ---------------- P6: final norm + head ----------------
        h3n = big.tile([128, C, T], F16, tag="X", name="h3n")
        with ExitStack() as s6:
            wsl = s6.enter_context(tc.tile_pool(name="wsl6", bufs=3))
            st = s6.enter_context(tc.tile_pool(name="st6", bufs=2))
            ps = s6.enter_context(tc.tile_pool(name="ps6", bufs=1, space="PSUM"))
            for c in range(C):
                h3t = st.tile([128, T], F32, tag="h3in")
                nc.sync.dma_start(h3t[:], h3_d[c])
                nc.vector.tensor_tensor(h3n[:, c, :], h3t[:], s_rep[:], AL.mult)
            h3n_r = h3n[:].rearrange("p c (s n) -> p c n s", n=N)
            psz = ps.tile([SEQ, FC], F32, tag="z")
            for t in range(N):
                slab = wsl.tile([128, C, FC], F16, tag="wfc")
                nc.sync.dma_start(slab[:], d["wfc"][t])
                for dc in range(C):
                    nc.tensor.matmul(psz[:], h3n_r[:, dc, t, :], slab[:, dc, :],
                                     start=(t == 0 and dc == 0),
                                     stop=(t == N - 1 and dc == C - 1))
            z1 = st.tile([SEQ, FC], F32, tag="z1")
            nc.vector.tensor_add(z1[:], psz[:], bfc_sb[:])
            zl = st.tile([SEQ, FC], F16, tag="zl")
            nc.scalar.activation(zl[:], z1[:], AF.Lrelu, alpha=0.01)
            pzt = ps.tile([FC, SEQ], F16, tag="zt")
            nc.tensor.transpose(pzt[:], zl[:], ident[0:SEQ, 0:SEQ])
            zT = st.tile([FC, SEQ], F16, tag="zT")
            nc.vector.tensor_copy(zT[:], pzt[:])
            ps2o = ps.tile([PRED, SEQ], F32, tag="o2")
            nc.tensor.matmul(ps2o[:], wout_sb[:], zT[:], start=True, stop=True)
            osb = st.tile([PRED, SEQ], F32, tag="osb")
            nc.vector.tensor_tensor(
                osb[:], ps2o[:],
                bout_sb[:, 0:1].to_broadcast((PRED, SEQ)), AL.add)
            nc.sync.dma_start(out_d[:], osb[:])

    nc.finalize()
    return nc


# ---------------- host side ----------------

def _prep_weights(inputs):
    f16 = np.float16
    anw = np.asarray(inputs["attn_norm_w"], np.float32)
    mnw = np.asarray(inputs["mlp_norm_w"], np.float32)
    fnw = np.asarray(inputs["final_norm_w"], np.float32)

    def swz(wT, nf):  # [Din, nf*128] -> [nf, 128ci, C, 128m]
        return np.ascontiguousarray(
            wT.reshape(C, 128, nf, 128).transpose(2, 1, 0, 3).astype(f16))

    w = {}
    w["w_in"] = np.zeros((128, C, 128), f16)
    w["w_in"][:P_PATCH] = (np.asarray(inputs["W_in"], np.float32).T
                           .reshape(P_PATCH, C, 128).astype(f16))
    w["b_in"] = np.ascontiguousarray(
        np.asarray(inputs["b_in"], np.float32).reshape(C, 128).T)
    w["wq"] = swz((np.asarray(inputs["Wq"], np.float32) * anw[None, :]).T, H)
    w["wk"] = swz((np.asarray(inputs["Wk"], np.float32) * anw[None, :]).T, KV)
    w["wv"] = swz((np.asarray(inputs["Wv"], np.float32) * anw[None, :]).T, KV)
    w["wo"] = swz(np.asarray(inputs["Wo"], np.float32).T, C)
    w["wg"] = swz((np.asarray(inputs["Wg"], np.float32) * mnw[None, :]).T, JC)
    w["wu"] = swz((np.asarray(inputs["Wu"], np.float32) * mnw[None, :]).T, JC)
    wdT = np.asarray(inputs["Wd"], np.float32).T          # [DFF, D]
    wd5 = wdT.reshape(NG, JG, 128, C, 128)                # [g, jj, ji, f, m]
    w["wd"] = np.ascontiguousarray(wd5.transpose(0, 3, 2, 1, 4).astype(f16))
    wfcT = (np.asarray(inputs["W_fc"], np.float32).reshape(FC, N, D)
            * fnw[None, None, :]).reshape(FC, N * D).T    # [N*D, FC]
    w["wfc"] = np.ascontiguousarray(
        wfcT.reshape(N, C, 128, FC).transpose(0, 2, 1, 3).astype(f16))
    w["wout"] = np.ascontiguousarray(
        np.asarray(inputs["W_out"], np.float32).T.astype(f16))
    w["b_fc"] = np.asarray(inputs["b_fc"], np.float32).reshape(1, FC).copy()
    w["b_out"] = np.asarray(inputs["b_out"], np.float32).reshape(PRED, 1).copy()

    inv_freq = 1.0 / (THETA ** (np.arange(0, HD, 2, dtype=np.float32) / HD))
    ang = np.arange(N, dtype=np.float32)[:, None] * inv_freq[None, :]
    cos_h = np.cos(ang).T.astype(np.float32)              # [64, N]
    sin_h = np.sin(ang).T.astype(np.float32)
    cos_t = np.concatenate([cos_h, cos_h], 0)
    sin_t = np.concatenate([-sin_h, sin_h], 0)            # sign-folded
    sc = 1.0 / math.sqrt(HD)
    w["cos_q"] = np.ascontiguousarray(cos_t * sc)
    w["sin_q"] = np.ascontiguousarray(sin_t * sc)
    w["cos_k"] = np.ascontiguousarray(cos_t)
    w["sin_k"] = np.ascontiguousarray(sin_t)
    kk = np.arange(N)[:, None]
    qq = np.arange(N)[None, :]
    w["mask"] = np.ascontiguousarray((kk <= qq).astype(f16))
    return w


_NC_CACHE = {}


def kernel(**inputs) -> np.ndarray:
    x = np.asarray(inputs["x"], np.float32)
    means = x.mean(axis=1, keepdims=True)                 # (16, 1, 7)
    stdev = np.sqrt(x.var(axis=1) + EPS)                  # (16, 7)
    xn = (x - means) / stdev[:, None, :]
    xt = xn.transpose(0, 2, 1).reshape(B * M, L)
    xp = np.concatenate([xt, np.repeat(xt[:, -1:], STRIDE, 1)], 1)
    idx = np.arange(N)[:, None] * STRIDE + np.arange(P_PATCH)[None, :]
    patches = xp[:, idx]                                  # (112, 64, 16)

    w = _prep_weights(inputs)

    if "nc" not in _NC_CACHE:
        _NC_CACHE["nc"] = build_nc()
    nc = _NC_CACHE["nc"]

    in_maps = []
    for core in range(NCORES):
        pc = patches[core * SEQ:(core + 1) * SEQ]
        pt = np.zeros((128, T), np.float16)
        pt[:P_PATCH] = pc.reshape(T, P_PATCH).T.astype(np.float16)
        m = dict(w)
        m["patches"] = pt
        in_maps.append(m)

    res = bass_utils.run_bass_kernel_spmd(nc, in_maps, core_ids=list(range(NCORES)))

    out = np.zeros((B, PRED, M), np.float32)
    for core in range(NCORES):
        oc = res.results[core]["out"]                     # (96, 14)
        for sl in range(SEQ):
            s = core * SEQ + sl
            b, mi = divmod(s, M)
            out[b, :, mi] = oc[:, sl] * stdev[b, mi] + means[b, 0, mi]
    return out



# revision 19
# speedup vs baseline: 1.0466x; 1.0466x over previous
"""Trainium2 Bass kernel for nn_Llama3_68135361184133.

Strategy: pure data-parallel over the 112 (b,m) sequences -> 14 seqs (896
tokens) per core, 8 cores, no collectives.  Compute dtype fp16 (weights +
matmul activations), fp32 accumulation.  Activations are feature-major
([128 chan, chunk, token]) so every matmul contracts over the partition
axis.  Per-token RMS scales are applied as post-matmul multiplies
(linearity) instead of materializing normalized copies.

v2 changes vs baseline:
 - patch-embed bias folded into the matmul (ones row in patches).
 - sum-of-squares via ACT Square + DVE accumulate (off the PE) in P4/P5.
 - RMS 1/sqrt via ACT Rsqrt; s_rep broadcast via 1-partition matmul
   (no DRAM bounce).
 - attention softmax batched per sequence: ACT Exp per group, one DVE
   mask multiply, ACT Reciprocal on [1,2048] (kills the 185us DVE
   reciprocal hotspot).
 - residual stream staged f16; h2/h3 kept in SBUF (no DRAM round trip),
   h3 computed in place over h2.
 - MLP g/u/d weight slabs loaded once (both token blocks per slab).
 - W_fc slabs prefetched during P5 via a persistent pool.
"""

import math
from contextlib import ExitStack

import numpy as np

import concourse.bass as bass
from concourse import bacc
import concourse.mybir as mybir
import concourse.tile as tile
from concourse import bass_utils
from concourse.masks import make_identity

F16 = mybir.dt.float16
F32 = mybir.dt.float32
AL = mybir.AluOpType
AF = mybir.ActivationFunctionType

B, L, M = 16, 512, 7
P_PATCH, STRIDE = 16, 8
N = 64
D, H, KV, HD, DFF = 4096, 32, 8, 128, 14336
FC, PRED = 128, 96
THETA = 500000.0
EPS = 1e-5

NCORES = 8
SEQ = 14
T = SEQ * N                # 896
C = D // 128               # 32
JC = DFF // 128            # 112
TB = 448
NTB = T // TB              # 2
JG = 14
NG = JC // JG              # 8
SPT = TB // N              # 7 sequences per token-block


def build_nc():
    nc = bacc.Bacc("TRN2")

    d = {}
    d["patches"] = nc.dram_tensor("patches", [128, T], F16, kind="ExternalInput")
    d["w_in"] = nc.dram_tensor("w_in", [128, C, 128], F16, kind="ExternalInput")
    d["wq"] = nc.dram_tensor("wq", [H, 128, C, 128], F16, kind="ExternalInput")
    d["wk"] = nc.dram_tensor("wk", [KV, 128, C, 128], F16, kind="ExternalInput")
    d["wv"] = nc.dram_tensor("wv", [KV, 128, C, 128], F16, kind="ExternalInput")
    d["wo"] = nc.dram_tensor("wo", [C, 128, C, 128], F16, kind="ExternalInput")
    d["wg"] = nc.dram_tensor("wg", [JC, 128, C, 128], F16, kind="ExternalInput")
    d["wu"] = nc.dram_tensor("wu", [JC, 128, C, 128], F16, kind="ExternalInput")
    d["wd"] = nc.dram_tensor("wd", [NG, C, 128, JG, 128], F16, kind="ExternalInput")
    d["wfc"] = nc.dram_tensor("wfc", [N, 128, C, FC], F16, kind="ExternalInput")
    d["wout"] = nc.dram_tensor("wout", [FC, PRED], F16, kind="ExternalInput")
    d["b_fc"] = nc.dram_tensor("b_fc", [1, FC], F32, kind="ExternalInput")
    d["b_out"] = nc.dram_tensor("b_out", [PRED, 1], F32, kind="ExternalInput")
    d["cos_q"] = nc.dram_tensor("cos_q", [128, N], F32, kind="ExternalInput")
    d["sin_q"] = nc.dram_tensor("sin_q", [128, N], F32, kind="ExternalInput")
    d["cos_k"] = nc.dram_tensor("cos_k", [128, N], F32, kind="ExternalInput")
    d["sin_k"] = nc.dram_tensor("sin_k", [128, N], F32, kind="ExternalInput")
    d["mask"] = nc.dram_tensor("mask", [N, N], F16, kind="ExternalInput")
    out_d = nc.dram_tensor("out", [PRED, SEQ], F32, kind="ExternalOutput")

    def tb(th):
        return slice(th * TB, (th + 1) * TB)

    with tile.TileContext(nc) as tc, ExitStack() as ctx:
        consts = ctx.enter_context(tc.tile_pool(name="consts", bufs=1))
        dram = ctx.enter_context(tc.tile_pool(name="dram", bufs=1, space="DRAM"))
        srep_p = ctx.enter_context(tc.tile_pool(name="srep", bufs=1))
        big = ctx.enter_context(tc.tile_pool(name="big", bufs=1))

        h1_d = dram.tile([C, 128, T], F16)

        patches_sb = consts.tile([128, T], F16)
        nc.sync.dma_start(patches_sb[:], d["patches"][:])
        cosq = consts.tile([128, N], F32)
        nc.sync.dma_start(cosq[:], d["cos_q"][:])
        sinq = consts.tile([128, N], F32)
        nc.sync.dma_start(sinq[:], d["sin_q"][:])
        cosk = consts.tile([128, N], F32)
        nc.sync.dma_start(cosk[:], d["cos_k"][:])
        sink = consts.tile([128, N], F32)
        nc.sync.dma_start(sink[:], d["sin_k"][:])
        mask_sb = consts.tile([N, N], F16)
        nc.sync.dma_start(mask_sb[:], d["mask"][:])
        ones128 = consts.tile([128, 1], F16)
        nc.vector.memset(ones128[:], 1.0)
        ones128f = consts.tile([128, 1], F32)
        nc.vector.memset(ones128f[:], 1.0)
        ones1 = consts.tile([1, 128], F16)
        nc.vector.memset(ones1[:], 1.0)
        ones1f = consts.tile([1, 128], F32)
        nc.vector.memset(ones1f[:], 1.0)
        ident = consts.tile([128, 128], F16)
        make_identity(nc, ident[:])
        wout_sb = consts.tile([FC, PRED], F16)
        nc.sync.dma_start(wout_sb[:], d["wout"][:])
        bfc_sb = consts.tile([SEQ, FC], F32)
        nc.sync.dma_start(bfc_sb[:], d["b_fc"][:].to_broadcast((SEQ, FC)))
        bout_sb = consts.tile([PRED, 1], F32)
        nc.sync.dma_start(bout_sb[:], d["b_out"][:])
        eps_sb = consts.tile([1, 1], F32)
        nc.vector.memset(eps_sb[:], EPS)

        s1_rep = srep_p.tile([128, T], F32, tag="s1")
        s2_rep = srep_p.tile([128, T], F32, tag="s2")
        s3_rep = srep_p.tile([128, T], F32, tag="s3")

        def rms_to_srep(sqacc_ap, dst_slice, st_pool, psr_pool):
            """dst = broadcast128(rsqrt(colsum(sqacc)/D + eps)).

            rsqrt as exp(-0.5*ln(x)): AF.Rsqrt is blocked by bass, and Ln/Exp
            share one ACT table set."""
            pss = psr_pool.tile([1, TB], F32, tag="ss")
            nc.tensor.matmul(pss[:], ones128f[:], sqacc_ap, start=True, stop=True)
            lnv = st_pool.tile([1, TB], F32, tag="lnv")
            nc.scalar.activation(lnv[:], pss[:], AF.Ln, bias=eps_sb[:],
                                 scale=1.0 / D)
            svec = st_pool.tile([1, TB], F32, tag="svec")
            nc.scalar.activation(svec[:], lnv[:], AF.Exp, scale=-0.5)
            psb = psr_pool.tile([128, TB], F32, tag="bc")
            nc.tensor.matmul(psb[:], ones1f[:], svec[:], start=True, stop=True)
            nc.scalar.copy(dst_slice, psb[:])

        # slot X: h1f16 -> o_T -> delta16 ; slot Y: q_T -> h2f16 (-> h3n in place)
        h1f16 = big.tile([128, C, T], F16, tag="X", name="h1f16")

        # ---------------- P1: patch embed ----------------
        with nc.named_scope("P1"), ExitStack() as s1:
            wsl = s1.enter_context(tc.tile_pool(name="wsl1", bufs=1))
            st = s1.enter_context(tc.tile_pool(name="st1", bufs=3))
            sqp = s1.enter_context(tc.tile_pool(name="sq1", bufs=2))
            ps = s1.enter_context(tc.tile_pool(name="ps1", bufs=2, space="PSUM"))
            psr = s1.enter_context(tc.tile_pool(name="psr1", bufs=2, space="PSUM"))
            w_in_sb = wsl.tile([128, C, 128], F16, tag="win")
            nc.sync.dma_start(w_in_sb[:], d["w_in"][:])
            for th in range(NTB):
                sqacc = sqp.tile([128, TB], F32, tag="acc")
                for c in range(C):
                    pse = ps.tile([128, TB], F32, tag="mm")
                    nc.tensor.matmul(pse[:], w_in_sb[:, c, :], patches_sb[:, tb(th)],
                                     start=True, stop=True)
                    nc.scalar.copy(h1f16[:, c, tb(th)], pse[:])
                    sq = st.tile([128, TB], F16, tag="sq")
                    nc.scalar.activation(sq[:], pse[:], AF.Square)
                    if c == 0:
                        nc.vector.tensor_copy(sqacc[:], sq[:])
                    else:
                        nc.vector.tensor_add(sqacc[:], sqacc[:], sq[:])
                rms_to_srep(sqacc[:], s1_rep[:, tb(th)], st, psr)
            for c in range(C):
                nc.sync.dma_start(h1_d[c], h1f16[:, c, :])

        # ---------------- P2 + P3 ----------------
        q_T = big.tile([128, H, T], F16, tag="Y", name="q_T")
        with ExitStack() as s23:
            kv = s23.enter_context(tc.tile_pool(name="kv", bufs=1))
            k_T = kv.tile([128, KV, T], F16, tag="k")
            v_N = kv.tile([64, SEQ, KV, HD], F16, tag="v")

            with nc.named_scope("P2"), ExitStack() as s2:
                wsl = s2.enter_context(tc.tile_pool(name="wsl2", bufs=3))
                rt = s2.enter_context(tc.tile_pool(name="rt2", bufs=2))
                ps = s2.enter_context(tc.tile_pool(name="ps2", bufs=2, space="PSUM"))
                pst = s2.enter_context(tc.tile_pool(name="pst2", bufs=2, space="PSUM"))

                def proj_rope(src, nf, dst, cos_t, sin_t):
                    for f in range(nf):
                        slab = wsl.tile([128, C, 128], F16, tag="w")
                        nc.sync.dma_start(slab[:], src[f])
                        for th in range(NTB):
                            p = ps.tile([128, TB], F32, tag="mm")
                            for c in range(C):
                                nc.tensor.matmul(p[:], slab[:, c, :],
                                                 h1f16[:, c, tb(th)],
                                                 start=(c == 0), stop=(c == C - 1))
                            pv = p[:].rearrange("p (s n) -> p s n", n=N)
                            t1 = rt.tile([128, TB], F32, tag="t1")
                            t2 = rt.tile([128, TB], F32, tag="t2")
                            t1v = t1[:].rearrange("p (s n) -> p s n", n=N)
                            t2v = t2[:].rearrange("p (s n) -> p s n", n=N)
                            cb = cos_t[:, None, :].to_broadcast((128, SPT, N))
                            nc.vector.tensor_tensor(t1v, pv, cb, AL.mult)
                            slo = sin_t[0:64][:, None, :].to_broadcast((64, SPT, N))
                            shi = sin_t[64:128][:, None, :].to_broadcast((64, SPT, N))
                            nc.vector.tensor_tensor(t2v[0:64], pv[64:128], slo, AL.mult)
                            nc.vector.tensor_tensor(t2v[64:128], pv[0:64], shi, AL.mult)
                            nc.vector.tensor_add(t1[:], t1[:], t2[:])
                            nc.vector.tensor_tensor(dst[:, f, tb(th)], t1[:],
                                                    s1_rep[:, tb(th)], AL.mult)

                proj_rope(d["wq"], H, q_T, cosq, sinq)
                proj_rope(d["wk"], KV, k_T, cosk, sink)

                for f in range(KV):
                    slab = wsl.tile([128, C, 128], F16, tag="w")
                    nc.sync.dma_start(slab[:], d["wv"][f])
                    for th in range(NTB):
                        p = ps.tile([128, TB], F32, tag="mm")
                        for c in range(C):
                            nc.tensor.matmul(p[:], slab[:, c, :], h1f16[:, c, tb(th)],
                                             start=(c == 0), stop=(c == C - 1))
                        vt = rt.tile([128, TB], F16, tag="vt")
                        nc.vector.tensor_tensor(vt[:], p[:], s1_rep[:, tb(th)], AL.mult)
                        for si in range(SPT):
                            s = th * SPT + si
                            ptr = pst.tile([N, 128], F16, tag="tr")
                            nc.tensor.transpose(ptr[:], vt[:, si * N:(si + 1) * N],
                                                ident[:])
                            nc.vector.tensor_copy(v_N[:, s, f, :], ptr[:])

            # P3: attention (slot X reused: o_T), batched softmax per sequence
            o_T = big.tile([128, C, T], F16, tag="X", name="o_T")
            with nc.named_scope("P3"), ExitStack() as s3:
                at = s3.enter_context(tc.tile_pool(name="at3", bufs=2))
                lnp = s3.enter_context(tc.tile_pool(name="ln3", bufs=1))
                r2p = s3.enter_context(tc.tile_pool(name="r23", bufs=2))
                psc = s3.enter_context(tc.tile_pool(name="psc3", bufs=2, space="PSUM"))
                psd = s3.enter_context(tc.tile_pool(name="psd3", bufs=2, space="PSUM"))
                psb = s3.enter_context(tc.tile_pool(name="psb3", bufs=2, space="PSUM"))
                pso = s3.enter_context(tc.tile_pool(name="pso3", bufs=2, space="PSUM"))
                for s in range(SEQ):
                    sl = slice(s * N, (s + 1) * N)
                    pt_all = at.tile([N, KV, 4, N], F16, tag="pt")
                    dn_all = at.tile([1, KV, 4 * N], F16, tag="dn")
                    lnd = lnp.tile([1, KV * 4 * N], F32, tag="lnd")
                    rv = at.tile([1, KV, 4 * N], F16, tag="rv")
                    for g in range(KV):
                        qs = q_T[:, 4 * g:4 * g + 4, sl]
                        ks = k_T[:, g, sl]
                        psp = psc.tile([N, 4, N], F32, tag="pp")
                        nc.tensor.matmul(psp[:], ks, qs, start=True, stop=True)
                        nc.scalar.activation(pt_all[:, g], psp[:], AF.Exp)
                    ptv = pt_all[:].rearrange("k g h n -> k (g h) n")
                    mb = mask_sb[:, None, :].to_broadcast((N, KV * 4, N))
                    nc.vector.tensor_tensor(ptv, ptv, mb, AL.mult)
                    for g in range(KV):
                        pd = psd.tile([1, 4 * N], F32, tag="pd")
                        nc.tensor.matmul(pd[:], ones128[0:N, :],
                                         pt_all[:, g].rearrange("k h n -> k (h n)"),
                                         start=True, stop=True)
                        nc.vector.tensor_copy(dn_all[:, g], pd[:])
                    # 1/x as exp(-ln(x)): Ln and Exp share one ACT table set,
                    # Reciprocal does not (28 table switches otherwise).
                    nc.scalar.activation(
                        lnd[:], dn_all[:].rearrange("o g x -> o (g x)"), AF.Ln)
                    nc.scalar.activation(
                        rv[:].rearrange("o g x -> o (g x)"), lnd[:],
                        AF.Exp, scale=-1.0)
                    for g2 in range(KV // 2):
                        pb = psb.tile([128, 2, 4 * N], F32, tag="pb")
                        nc.tensor.matmul(pb[:], ones1[:],
                                         rv[:, 2 * g2:2 * g2 + 2],
                                         start=True, stop=True)
                        r2 = r2p.tile([128, 2, 4, N], F16, tag="r2")
                        nc.scalar.copy(r2[:].rearrange("p g h n -> p (g h n)"),
                                       pb[:].rearrange("p g x -> p (g x)"))
                        for gi in range(2):
                            g = 2 * g2 + gi
                            vs = v_N[:, s, g, :]
                            po = pso.tile([128, 4 * N], F32, tag="po")
                            nc.tensor.matmul(po[:], vs,
                                             pt_all[:, g].rearrange("k h n -> k (h n)"),
                                             start=True, stop=True)
                            ov = o_T[:, 4 * g:4 * g + 4, sl]
                            po3 = po[:].rearrange("p (h n) -> p h n", n=N)
                            nc.vector.tensor_tensor(ov, po3, r2[:, gi], AL.mult)

        # ---------------- P4: Wo + residual ----------------
        h2f16 = big.tile([128, C, T], F16, tag="Y", name="h2f16")
        with nc.named_scope("P4"), ExitStack() as s4:
            wsl = s4.enter_context(tc.tile_pool(name="wsl4", bufs=2))
            st = s4.enter_context(tc.tile_pool(name="st4", bufs=3))
            sqp = s4.enter_context(tc.tile_pool(name="sq4", bufs=2))
            ps = s4.enter_context(tc.tile_pool(name="ps4", bufs=2, space="PSUM"))
            psr = s4.enter_context(tc.tile_pool(name="psr4", bufs=2, space="PSUM"))
            sqaccs = [sqp.tile([128, TB], F32, tag=f"acc{th}", name=f"sqacc{th}")
                      for th in range(NTB)]
            for f in range(C):
                slab = wsl.tile([128, C, 128], F16, tag="w")
                nc.sync.dma_start(slab[:], d["wo"][f])
                for th in range(NTB):
                    p = ps.tile([128, TB], F32, tag="mm")
                    for c in range(C):
                        nc.tensor.matmul(p[:], slab[:, c, :], o_T[:, c, tb(th)],
                                         start=(c == 0), stop=(c == C - 1))
                    h1t = st.tile([128, TB], F16, tag="h1t")
                    nc.sync.dma_start(h1t[:], h1_d[f, :, tb(th)])
                    nc.vector.tensor_add(h2f16[:, f, tb(th)], p[:], h1t[:])
                    sq = st.tile([128, TB], F16, tag="sq")
                    nc.scalar.activation(sq[:], h2f16[:, f, tb(th)], AF.Square)
                    if f == 0:
                        nc.vector.tensor_copy(sqaccs[th][:], sq[:])
                    else:
                        nc.vector.tensor_add(sqaccs[th][:], sqaccs[th][:], sq[:])
            for th in range(NTB):
                rms_to_srep(sqaccs[th][:], s2_rep[:, tb(th)], st, psr)

        # ---------------- P5: SwiGLU MLP ----------------
        delta16 = big.tile([128, C, T], F16, tag="X", name="delta16")
        with nc.named_scope("P5"), ExitStack() as s5:
            wsl = s5.enter_context(tc.tile_pool(name="wsl5", bufs=2))
            dslp = s5.enter_context(tc.tile_pool(name="dsl5", bufs=2))
            agp = s5.enter_context(tc.tile_pool(name="ag5", bufs=1))
            mt = s5.enter_context(tc.tile_pool(name="mt5", bufs=3))
            st = s5.enter_context(tc.tile_pool(name="st5", bufs=3))
            sqp = s5.enter_context(tc.tile_pool(name="sq5", bufs=2))
            ps = s5.enter_context(tc.tile_pool(name="ps5", bufs=2, space="PSUM"))
            psr = s5.enter_context(tc.tile_pool(name="psr5", bufs=1, space="PSUM"))
            a_gs = [agp.tile([128, JG, TB], F16, tag=f"ag{th}", name=f"a_g{th}")
                    for th in range(NTB)]
            for g in range(NG):
                for jj in range(JG):
                    j = g * JG + jj
                    gs = wsl.tile([128, C, 128], F16, tag="wgu")
                    nc.sync.dma_start(gs[:], d["wg"][j])
                    us = wsl.tile([128, C, 128], F16, tag="wgu")
                    nc.sync.dma_start(us[:], d["wu"][j])
                    for th in range(NTB):
                        psg = ps.tile([128, TB], F32, tag="g")
                        psu = ps.tile([128, TB], F32, tag="u")
                        for c in range(C):
                            nc.tensor.matmul(psg[:], gs[:, c, :], h2f16[:, c, tb(th)],
                                             start=(c == 0), stop=(c == C - 1))
                        for c in range(C):
                            nc.tensor.matmul(psu[:], us[:, c, :], h2f16[:, c, tb(th)],
                                             start=(c == 0), stop=(c == C - 1))
                        t1 = mt.tile([128, TB], F16, tag="gt")
                        nc.vector.tensor_tensor(t1[:], psg[:], s2_rep[:, tb(th)],
                                                AL.mult)
                        sg = mt.tile([128, TB], F16, tag="sg")
                        nc.scalar.activation(sg[:], t1[:], AF.Silu)
                        nc.vector.tensor_tensor(a_gs[th][:, jj, :], sg[:], psu[:],
                                                AL.mult)
                for f in range(C):
                    dsl = dslp.tile([128, JG, 128], F16, tag="wd")
                    nc.sync.dma_start(dsl[:], d["wd"][g, f])
                    for th in range(NTB):
                        pd = ps.tile([128, TB], F32, tag="dd")
                        for jj in range(JG):
                            nc.tensor.matmul(pd[:], dsl[:, jj, :], a_gs[th][:, jj, :],
                                             start=(jj == 0), stop=(jj == JG - 1))
                        if g == 0:
                            nc.vector.tensor_copy(delta16[:, f, tb(th)], pd[:])
                        else:
                            nc.vector.tensor_add(delta16[:, f, tb(th)],
                                                 delta16[:, f, tb(th)], pd[:])
            # h3 = h2 + delta*s2 (in place over h2f16), then final norm scale
            for th in range(NTB):
                sqacc = sqp.tile([128, TB], F32, tag="acc")
                for f in range(C):
                    dt = st.tile([128, TB], F16, tag="dt")
                    nc.vector.tensor_tensor(dt[:], delta16[:, f, tb(th)],
                                            s2_rep[:, tb(th)], AL.mult)
                    nc.vector.tensor_add(h2f16[:, f, tb(th)], h2f16[:, f, tb(th)],
                                         dt[:])
                    sq = st.tile([128, TB], F16, tag="sq")
                    nc.scalar.activation(sq[:], h2f16[:, f, tb(th)], AF.Square)
                    if f == 0:
                        nc.vector.tensor_copy(sqacc[:], sq[:])
                    else:
                        nc.vector.tensor_add(sqacc[:], sqacc[:], sq[:])
                rms_to_srep(sqacc[:], s3_rep[:, tb(th)], st, psr)
                for f in range(C):
                    nc.vector.tensor_tensor(h2f16[:, f, tb(th)],
                                            h2f16[:, f, tb(th)],
                                            s3_rep[:, tb(th)], AL.mult)

        # ---------------- P6: head ----------------
        with nc.named_scope("P6"), ExitStack() as s6:
            st = s6.enter_context(tc.tile_pool(name="st6", bufs=2))
            wfcp = s6.enter_context(tc.tile_pool(name="wfcp", bufs=4))
            ps = s6.enter_context(tc.tile_pool(name="ps6", bufs=1, space="PSUM"))
            h3n_r = h2f16[:].rearrange("p c (s n) -> p c n s", n=N)
            psz = ps.tile([SEQ, FC], F32, tag="z")
            for t in range(N):
                slab = wfcp.tile([128, C, FC], F16, tag="wfc")
                nc.sync.dma_start(slab[:], d["wfc"][t])
                for dc in range(C):
                    nc.tensor.matmul(psz[:], h3n_r[:, dc, t, :], slab[:, dc, :],
                                     start=(t == 0 and dc == 0),
                                     stop=(t == N - 1 and dc == C - 1))
            z1 = st.tile([SEQ, FC], F32, tag="z1")
            nc.vector.tensor_add(z1[:], psz[:], bfc_sb[:])
            zl = st.tile([SEQ, FC], F16, tag="zl")
            nc.scalar.activation(zl[:], z1[:], AF.Lrelu, alpha=0.01)
            pzt = ps.tile([FC, SEQ], F16, tag="zt")
            nc.tensor.transpose(pzt[:], zl[:], ident[0:SEQ, 0:SEQ])
            zT = st.tile([FC, SEQ], F16, tag="zT")
            nc.vector.tensor_copy(zT[:], pzt[:])
            ps2o = ps.tile([PRED, SEQ], F32, tag="o2")
            nc.tensor.matmul(ps2o[:], wout_sb[:], zT[:], start=True, stop=True)
            osb = st.tile([PRED, SEQ], F32, tag="osb")
            nc.vector.tensor_tensor(
                osb[:], ps2o[:],
                bout_sb[:, 0:1].to_broadcast((PRED, SEQ)), AL.add)
            nc.sync.dma_start(out_d[:], osb[:])

    nc.finalize()
    return nc


# ---------------- host side ----------------

def _prep_weights(inputs):
    f16 = np.float16
    anw = np.asarray(inputs["attn_norm_w"], np.float32)
    mnw = np.asarray(inputs["mlp_norm_w"], np.float32)
    fnw = np.asarray(inputs["final_norm_w"], np.float32)

    def swz(wT, nf):  # [Din, nf*128] -> [nf, 128ci, C, 128m]
        return np.ascontiguousarray(
            wT.reshape(C, 128, nf, 128).transpose(2, 1, 0, 3).astype(f16))

    w = {}
    w["w_in"] = np.zeros((128, C, 128), f16)
    w["w_in"][:P_PATCH] = (np.asarray(inputs["W_in"], np.float32).T
                           .reshape(P_PATCH, C, 128).astype(f16))
    w["w_in"][P_PATCH] = (np.asarray(inputs["b_in"], np.float32)
                          .reshape(C, 128).astype(f16))
    w["wq"] = swz((np.asarray(inputs["Wq"], np.float32) * anw[None, :]).T, H)
    w["wk"] = swz((np.asarray(inputs["Wk"], np.float32) * anw[None, :]).T, KV)
    w["wv"] = swz((np.asarray(inputs["Wv"], np.float32) * anw[None, :]).T, KV)
    w["wo"] = swz(np.asarray(inputs["Wo"], np.float32).T, C)
    w["wg"] = swz((np.asarray(inputs["Wg"], np.float32) * mnw[None, :]).T, JC)
    w["wu"] = swz((np.asarray(inputs["Wu"], np.float32) * mnw[None, :]).T, JC)
    wdT = np.asarray(inputs["Wd"], np.float32).T          # [DFF, D]
    wd5 = wdT.reshape(NG, JG, 128, C, 128)                # [g, jj, ji, f, m]
    w["wd"] = np.ascontiguousarray(wd5.transpose(0, 3, 2, 1, 4).astype(f16))
    wfcT = (np.asarray(inputs["W_fc"], np.float32).reshape(FC, N, D)
            * fnw[None, None, :]).reshape(FC, N * D).T    # [N*D, FC]
    w["wfc"] = np.ascontiguousarray(
        wfcT.reshape(N, C, 128, FC).transpose(0, 2, 1, 3).astype(f16))
    w["wout"] = np.ascontiguousarray(
        np.asarray(inputs["W_out"], np.float32).T.astype(f16))
    w["b_fc"] = np.asarray(inputs["b_fc"], np.float32).reshape(1, FC).copy()
    w["b_out"] = np.asarray(inputs["b_out"], np.float32).reshape(PRED, 1).copy()

    inv_freq = 1.0 / (THETA ** (np.arange(0, HD, 2, dtype=np.float32) / HD))
    ang = np.arange(N, dtype=np.float32)[:, None] * inv_freq[None, :]
    cos_h = np.cos(ang).T.astype(np.float32)              # [64, N]
    sin_h = np.sin(ang).T.astype(np.float32)
    cos_t = np.concatenate([cos_h, cos_h], 0)
    sin_t = np.concatenate([-sin_h, sin_h], 0)            # sign-folded
    sc = 1.0 / math.sqrt(HD)
    w["cos_q"] = np.ascontiguousarray(cos_t * sc)
    w["sin_q"] = np.ascontiguousarray(sin_t * sc)
    w["cos_k"] = np.ascontiguousarray(cos_t)
    w["sin_k"] = np.ascontiguousarray(sin_t)
    kk = np.arange(N)[:, None]
    qq = np.arange(N)[None, :]
    w["mask"] = np.ascontiguousarray((kk <= qq).astype(f16))
    return w


def _prep_patches(x):
    means = x.mean(axis=1, keepdims=True)                 # (16, 1, 7)
    stdev = np.sqrt(x.var(axis=1) + EPS)                  # (16, 7)
    xn = (x - means) / stdev[:, None, :]
    xt = xn.transpose(0, 2, 1).reshape(B * M, L)
    xp = np.concatenate([xt, np.repeat(xt[:, -1:], STRIDE, 1)], 1)
    idx = np.arange(N)[:, None] * STRIDE + np.arange(P_PATCH)[None, :]
    patches = xp[:, idx]                                  # (112, 64, 16)
    return patches, means, stdev


def _core_patch_tile(patches, core):
    pc = patches[core * SEQ:(core + 1) * SEQ]
    pt = np.zeros((128, T), np.float16)
    pt[:P_PATCH] = pc.reshape(T, P_PATCH).T.astype(np.float16)
    pt[P_PATCH] = 1.0                                     # bias row
    return pt


def make_in_maps(inputs):
    x = np.asarray(inputs["x"], np.float32)
    patches, means, stdev = _prep_patches(x)
    w = _prep_weights(inputs)
    in_maps = []
    for core in range(NCORES):
        m = dict(w)
        m["patches"] = _core_patch_tile(patches, core)
        in_maps.append(m)
    return in_maps, means, stdev


_NC_CACHE = {}


def get_nc():
    if "nc" not in _NC_CACHE:
        _NC_CACHE["nc"] = build_nc()
    return _NC_CACHE["nc"]


def kernel(**inputs) -> np.ndarray:
    in_maps, means, stdev = make_in_maps(inputs)
    nc = get_nc()
    res = bass_utils.run_bass_kernel_spmd(nc, in_maps, core_ids=list(range(NCORES)))

    out = np.zeros((B, PRED, M), np.float32)
    for core in range(NCORES):
        oc = res.results[core]["out"]                     # (96, 14)
        for sl in range(SEQ):
            s = core * SEQ + sl
            b, mi = divmod(s, M)
            out[b, :, mi] = oc[:, sl] * stdev[b, mi] + means[b, 0, mi]
    return out


# revision 24
# speedup vs baseline: 1.0537x; 1.0067x over previous
"""Trainium2 Bass kernel for nn_Llama3_68135361184133.

Strategy: pure data-parallel over the 112 (b,m) sequences -> 14 seqs (896
tokens) per core, 8 cores, no collectives.  Compute dtype fp16 (weights +
matmul activations), fp32 accumulation.  Activations are feature-major
([128 chan, chunk, token]) so every matmul contracts over the partition
axis.  Per-token RMS scales are applied as post-matmul multiplies
(linearity) instead of materializing normalized copies.

v2 changes vs baseline:
 - patch-embed bias folded into the matmul (ones row in patches).
 - sum-of-squares via ACT Square + DVE accumulate (off the PE) in P4/P5.
 - RMS 1/sqrt via ACT Rsqrt; s_rep broadcast via 1-partition matmul
   (no DRAM bounce).
 - attention softmax batched per sequence: ACT Exp per group, one DVE
   mask multiply, ACT Reciprocal on [1,2048] (kills the 185us DVE
   reciprocal hotspot).
 - residual stream staged f16; h2/h3 kept in SBUF (no DRAM round trip),
   h3 computed in place over h2.
 - MLP g/u/d weight slabs loaded once (both token blocks per slab).
 - W_fc slabs prefetched during P5 via a persistent pool.
"""

import math
import types
from contextlib import ExitStack

import numpy as np

import bass_rust as _bass_rust
import concourse.bass as bass
from concourse import bacc
import concourse.mybir as mybir
import concourse.tile as tile
from concourse import bass_utils
from concourse.hw_specs import get_activation_tables
from concourse.masks import make_identity

F16 = mybir.dt.float16
F32 = mybir.dt.float32
AL = mybir.AluOpType
AF = mybir.ActivationFunctionType

B, L, M = 16, 512, 7
P_PATCH, STRIDE = 16, 8
N = 64
D, H, KV, HD, DFF = 4096, 32, 8, 128, 14336
FC, PRED = 128, 96
THETA = 500000.0
EPS = 1e-5

NCORES = 8
SEQ = 14
T = SEQ * N                # 896
C = D // 128               # 32
JC = DFF // 128            # 112
TB = 448
NTB = T // TB              # 2
JG = 14
NG = JC // JG              # 8
SPT = TB // N              # 7 sequences per token-block


_ACT_SETS = ("natural_log_exp_and_others", "silu_and_others",
             "derivative_gelu_apprx_sigmoid_and_others")  # last: Lrelu (P6, 1x)


def _steered_act_table_loads(self):
    """Restrict the ACT table-load pass to two sets so each kernel phase
    stays in one table (first-fit over the canonical list would split
    Exp/Ln across sets -> 28 table switches in attention).  Set ids stay
    canonical act_info.json indices; unwanted sets are just emptied so
    first-fit never picks them.  All functions this kernel uses (Exp, Ln,
    Square, Copy, Lrelu -> natural_log_exp_and_others; Silu ->
    silu_and_others) are covered."""
    has_activation = any(
        isinstance(i, mybir.InstActivation)
        for b in self.main_func.blocks
        for i in b.instructions
    )
    if not has_activation:
        return
    tables = [
        (name, funcs if name in _ACT_SETS else set())
        for name, funcs in get_activation_tables(self.m.arch).items()
    ]
    _bass_rust.insert_act_table_loads(self, tables)


def build_nc():
    nc = bacc.Bacc("TRN2")
    nc.insert_act_table_loads = types.MethodType(_steered_act_table_loads, nc)

    d = {}
    d["patches"] = nc.dram_tensor("patches", [128, T], F16, kind="ExternalInput")
    d["w_in"] = nc.dram_tensor("w_in", [128, C, 128], F16, kind="ExternalInput")
    d["wq"] = nc.dram_tensor("wq", [H, 128, C, 128], F16, kind="ExternalInput")
    d["wk"] = nc.dram_tensor("wk", [KV, 128, C, 128], F16, kind="ExternalInput")
    d["wv"] = nc.dram_tensor("wv", [KV, 128, C, 128], F16, kind="ExternalInput")
    d["wo"] = nc.dram_tensor("wo", [C, 128, C, 128], F16, kind="ExternalInput")
    d["wg"] = nc.dram_tensor("wg", [JC, 128, C, 128], F16, kind="ExternalInput")
    d["wu"] = nc.dram_tensor("wu", [JC, 128, C, 128], F16, kind="ExternalInput")
    d["wd"] = nc.dram_tensor("wd", [NG, C, 128, JG, 128], F16, kind="ExternalInput")
    d["wfc"] = nc.dram_tensor("wfc", [N, 128, C, FC], F16, kind="ExternalInput")
    d["wout"] = nc.dram_tensor("wout", [FC, PRED], F16, kind="ExternalInput")
    d["b_fc"] = nc.dram_tensor("b_fc", [1, FC], F32, kind="ExternalInput")
    d["b_out"] = nc.dram_tensor("b_out", [PRED, 1], F32, kind="ExternalInput")
    d["cos_q"] = nc.dram_tensor("cos_q", [128, N], F32, kind="ExternalInput")
    d["sin_q"] = nc.dram_tensor("sin_q", [128, N], F32, kind="ExternalInput")
    d["cos_k"] = nc.dram_tensor("cos_k", [128, N], F32, kind="ExternalInput")
    d["sin_k"] = nc.dram_tensor("sin_k", [128, N], F32, kind="ExternalInput")
    d["mask"] = nc.dram_tensor("mask", [N, N], F16, kind="ExternalInput")
    out_d = nc.dram_tensor("out", [PRED, SEQ], F32, kind="ExternalOutput")

    def tb(th):
        return slice(th * TB, (th + 1) * TB)

    with tile.TileContext(nc) as tc, ExitStack() as ctx:
        consts = ctx.enter_context(tc.tile_pool(name="consts", bufs=1))
        dram = ctx.enter_context(tc.tile_pool(name="dram", bufs=1, space="DRAM"))
        srep_p = ctx.enter_context(tc.tile_pool(name="srep", bufs=1))
        big = ctx.enter_context(tc.tile_pool(name="big", bufs=1))

        h1_d = dram.tile([C, 128, T], F16)

        patches_sb = consts.tile([128, T], F16)
        nc.sync.dma_start(patches_sb[:], d["patches"][:])
        cosq = consts.tile([128, N], F32)
        nc.sync.dma_start(cosq[:], d["cos_q"][:])
        sinq = consts.tile([128, N], F32)
        nc.sync.dma_start(sinq[:], d["sin_q"][:])
        cosk = consts.tile([128, N], F32)
        nc.sync.dma_start(cosk[:], d["cos_k"][:])
        sink = consts.tile([128, N], F32)
        nc.sync.dma_start(sink[:], d["sin_k"][:])
        mask_sb = consts.tile([N, N], F16)
        nc.sync.dma_start(mask_sb[:], d["mask"][:])
        ones128 = consts.tile([128, 1], F16)
        nc.vector.memset(ones128[:], 1.0)
        ones128f = consts.tile([128, 1], F32)
        nc.vector.memset(ones128f[:], 1.0)
        ones1 = consts.tile([1, 128], F16)
        nc.vector.memset(ones1[:], 1.0)
        ones1f = consts.tile([1, 128], F32)
        nc.vector.memset(ones1f[:], 1.0)
        ident = consts.tile([128, 128], F16)
        make_identity(nc, ident[:])
        wout_sb = consts.tile([FC, PRED], F16)
        nc.sync.dma_start(wout_sb[:], d["wout"][:])
        bfc_sb = consts.tile([SEQ, FC], F32)
        nc.sync.dma_start(bfc_sb[:], d["b_fc"][:].to_broadcast((SEQ, FC)))
        bout_sb = consts.tile([PRED, 1], F32)
        nc.sync.dma_start(bout_sb[:], d["b_out"][:])
        eps_sb = consts.tile([1, 1], F32)
        nc.vector.memset(eps_sb[:], EPS)

        s1_rep = srep_p.tile([128, T], F32, tag="s1")
        s2_rep = srep_p.tile([128, T], F32, tag="s2")
        s3_rep = srep_p.tile([128, T], F32, tag="s3")

        def rms_to_srep(sqacc_ap, dst_slice, st_pool, psr_pool):
            """dst = broadcast128(rsqrt(colsum(sqacc)/D + eps)).

            rsqrt as exp(-0.5*ln(x)): AF.Rsqrt is blocked by bass, and Ln/Exp
            share one ACT table set."""
            pss = psr_pool.tile([1, TB], F32, tag="ss")
            nc.tensor.matmul(pss[:], ones128f[:], sqacc_ap, start=True, stop=True)
            lnv = st_pool.tile([1, TB], F32, tag="lnv")
            nc.scalar.activation(lnv[:], pss[:], AF.Ln, bias=eps_sb[:],
                                 scale=1.0 / D)
            svec = st_pool.tile([1, TB], F32, tag="svec")
            nc.scalar.activation(svec[:], lnv[:], AF.Exp, scale=-0.5)
            psb = psr_pool.tile([128, TB], F32, tag="bc")
            nc.tensor.matmul(psb[:], ones1f[:], svec[:], start=True, stop=True)
            nc.scalar.copy(dst_slice, psb[:])

        # slot X: h1f16 -> o_T -> delta16 ; slot Y: q_T -> h2f16 (-> h3n in place)
        h1f16 = big.tile([128, C, T], F16, tag="X", name="h1f16")

        # ---------------- P1: patch embed ----------------
        with nc.named_scope("P1"), ExitStack() as s1:
            wsl = s1.enter_context(tc.tile_pool(name="wsl1", bufs=1))
            st = s1.enter_context(tc.tile_pool(name="st1", bufs=3))
            sqp = s1.enter_context(tc.tile_pool(name="sq1", bufs=2))
            ps = s1.enter_context(tc.tile_pool(name="ps1", bufs=2, space="PSUM"))
            psr = s1.enter_context(tc.tile_pool(name="psr1", bufs=2, space="PSUM"))
            w_in_sb = wsl.tile([128, C, 128], F16, tag="win")
            nc.sync.dma_start(w_in_sb[:], d["w_in"][:])
            for th in range(NTB):
                sqacc = sqp.tile([128, TB], F32, tag="acc")
                for c in range(C):
                    pse = ps.tile([128, TB], F32, tag="mm")
                    nc.tensor.matmul(pse[:], w_in_sb[:, c, :], patches_sb[:, tb(th)],
                                     start=True, stop=True)
                    nc.scalar.copy(h1f16[:, c, tb(th)], pse[:])
                    sq = st.tile([128, TB], F16, tag="sq")
                    nc.scalar.activation(sq[:], pse[:], AF.Square)
                    if c == 0:
                        nc.vector.tensor_copy(sqacc[:], sq[:])
                    else:
                        nc.vector.tensor_add(sqacc[:], sqacc[:], sq[:])
                rms_to_srep(sqacc[:], s1_rep[:, tb(th)], st, psr)
            for c in range(C):
                nc.sync.dma_start(h1_d[c], h1f16[:, c, :])

        # ---------------- P2 + P3 ----------------
        q_T = big.tile([128, H, T], F16, tag="Y", name="q_T")
        with ExitStack() as s23:
            kv = s23.enter_context(tc.tile_pool(name="kv", bufs=1))
            k_T = kv.tile([128, KV, T], F16, tag="k")
            v_N = kv.tile([64, SEQ, KV, HD], F16, tag="v")

            with nc.named_scope("P2"), ExitStack() as s2:
                wsl = s2.enter_context(tc.tile_pool(name="wsl2", bufs=3))
                rt = s2.enter_context(tc.tile_pool(name="rt2", bufs=2))
                ps = s2.enter_context(tc.tile_pool(name="ps2", bufs=2, space="PSUM"))
                pst = s2.enter_context(tc.tile_pool(name="pst2", bufs=2, space="PSUM"))

                def proj_rope(src, nf, dst, cos_t, sin_t):
                    for f in range(nf):
                        slab = wsl.tile([128, C, 128], F16, tag="w")
                        nc.sync.dma_start(slab[:], src[f])
                        for th in range(NTB):
                            p = ps.tile([128, TB], F32, tag="mm")
                            for c in range(C):
                                nc.tensor.matmul(p[:], slab[:, c, :],
                                                 h1f16[:, c, tb(th)],
                                                 start=(c == 0), stop=(c == C - 1))
                            pv = p[:].rearrange("p (s n) -> p s n", n=N)
                            t1 = rt.tile([128, TB], F32, tag="t1")
                            t2 = rt.tile([128, TB], F32, tag="t2")
                            t1v = t1[:].rearrange("p (s n) -> p s n", n=N)
                            t2v = t2[:].rearrange("p (s n) -> p s n", n=N)
                            cb = cos_t[:, None, :].to_broadcast((128, SPT, N))
                            nc.vector.tensor_tensor(t1v, pv, cb, AL.mult)
                            slo = sin_t[0:64][:, None, :].to_broadcast((64, SPT, N))
                            shi = sin_t[64:128][:, None, :].to_broadcast((64, SPT, N))
                            nc.vector.tensor_tensor(t2v[0:64], pv[64:128], slo, AL.mult)
                            nc.vector.tensor_tensor(t2v[64:128], pv[0:64], shi, AL.mult)
                            nc.vector.tensor_add(t1[:], t1[:], t2[:])
                            nc.vector.tensor_tensor(dst[:, f, tb(th)], t1[:],
                                                    s1_rep[:, tb(th)], AL.mult)

                proj_rope(d["wq"], H, q_T, cosq, sinq)
                proj_rope(d["wk"], KV, k_T, cosk, sink)

                for f in range(KV):
                    slab = wsl.tile([128, C, 128], F16, tag="w")
                    nc.sync.dma_start(slab[:], d["wv"][f])
                    for th in range(NTB):
                        p = ps.tile([128, TB], F32, tag="mm")
                        for c in range(C):
                            nc.tensor.matmul(p[:], slab[:, c, :], h1f16[:, c, tb(th)],
                                             start=(c == 0), stop=(c == C - 1))
                        vt = rt.tile([128, TB], F16, tag="vt")
                        nc.vector.tensor_tensor(vt[:], p[:], s1_rep[:, tb(th)], AL.mult)
                        for si in range(SPT):
                            s = th * SPT + si
                            ptr = pst.tile([N, 128], F16, tag="tr")
                            nc.tensor.transpose(ptr[:], vt[:, si * N:(si + 1) * N],
                                                ident[:])
                            nc.vector.tensor_copy(v_N[:, s, f, :], ptr[:])

            # P3: attention (slot X reused: o_T), batched softmax per sequence
            o_T = big.tile([128, C, T], F16, tag="X", name="o_T")
            with nc.named_scope("P3"), ExitStack() as s3:
                at = s3.enter_context(tc.tile_pool(name="at3", bufs=2))
                lnp = s3.enter_context(tc.tile_pool(name="ln3", bufs=1))
                r2p = s3.enter_context(tc.tile_pool(name="r23", bufs=2))
                psc = s3.enter_context(tc.tile_pool(name="psc3", bufs=2, space="PSUM"))
                psd = s3.enter_context(tc.tile_pool(name="psd3", bufs=2, space="PSUM"))
                psb = s3.enter_context(tc.tile_pool(name="psb3", bufs=2, space="PSUM"))
                pso = s3.enter_context(tc.tile_pool(name="pso3", bufs=2, space="PSUM"))
                for s in range(SEQ):
                    sl = slice(s * N, (s + 1) * N)
                    pt_all = at.tile([N, KV, 4, N], F16, tag="pt")
                    dn_all = at.tile([1, KV, 4 * N], F16, tag="dn")
                    lnd = lnp.tile([1, KV * 4 * N], F32, tag="lnd")
                    rv = at.tile([1, KV, 4 * N], F16, tag="rv")
                    for g in range(KV):
                        qs = q_T[:, 4 * g:4 * g + 4, sl]
                        ks = k_T[:, g, sl]
                        psp = psc.tile([N, 4, N], F32, tag="pp")
                        nc.tensor.matmul(psp[:], ks, qs, start=True, stop=True)
                        nc.scalar.activation(pt_all[:, g], psp[:], AF.Exp)
                    ptv = pt_all[:].rearrange("k g h n -> k (g h) n")
                    mb = mask_sb[:, None, :].to_broadcast((N, KV * 4, N))
                    nc.vector.tensor_tensor(ptv, ptv, mb, AL.mult)
                    for g in range(KV):
                        pd = psd.tile([1, 4 * N], F32, tag="pd")
                        nc.tensor.matmul(pd[:], ones128[0:N, :],
                                         pt_all[:, g].rearrange("k h n -> k (h n)"),
                                         start=True, stop=True)
                        nc.vector.tensor_copy(dn_all[:, g], pd[:])
                    # 1/x as exp(-ln(x)): Ln and Exp share one ACT table set,
                    # Reciprocal does not (28 table switches otherwise).
                    nc.scalar.activation(
                        lnd[:], dn_all[:].rearrange("o g x -> o (g x)"), AF.Ln)
                    nc.scalar.activation(
                        rv[:].rearrange("o g x -> o (g x)"), lnd[:],
                        AF.Exp, scale=-1.0)
                    for g2 in range(KV // 2):
                        pb = psb.tile([128, 2, 4 * N], F32, tag="pb")
                        nc.tensor.matmul(pb[:], ones1[:],
                                         rv[:, 2 * g2:2 * g2 + 2],
                                         start=True, stop=True)
                        r2 = r2p.tile([128, 2, 4, N], F16, tag="r2")
                        nc.vector.tensor_copy(r2[:].rearrange("p g h n -> p (g h n)"),
                                              pb[:].rearrange("p g x -> p (g x)"))
                        for gi in range(2):
                            g = 2 * g2 + gi
                            vs = v_N[:, s, g, :]
                            po = pso.tile([128, 4 * N], F32, tag="po")
                            nc.tensor.matmul(po[:], vs,
                                             pt_all[:, g].rearrange("k h n -> k (h n)"),
                                             start=True, stop=True)
                            ov = o_T[:, 4 * g:4 * g + 4, sl]
                            po3 = po[:].rearrange("p (h n) -> p h n", n=N)
                            nc.vector.tensor_tensor(ov, po3, r2[:, gi], AL.mult)

        # ---------------- P4: Wo + residual ----------------
        h2f16 = big.tile([128, C, T], F16, tag="Y", name="h2f16")
        with nc.named_scope("P4"), ExitStack() as s4:
            wsl = s4.enter_context(tc.tile_pool(name="wsl4", bufs=2))
            st = s4.enter_context(tc.tile_pool(name="st4", bufs=3))
            sqp = s4.enter_context(tc.tile_pool(name="sq4", bufs=2))
            ps = s4.enter_context(tc.tile_pool(name="ps4", bufs=2, space="PSUM"))
            psr = s4.enter_context(tc.tile_pool(name="psr4", bufs=2, space="PSUM"))
            sqaccs = [sqp.tile([128, TB], F32, tag=f"acc{th}", name=f"sqacc{th}")
                      for th in range(NTB)]
            for f in range(C):
                slab = wsl.tile([128, C, 128], F16, tag="w")
                nc.sync.dma_start(slab[:], d["wo"][f])
                for th in range(NTB):
                    p = ps.tile([128, TB], F32, tag="mm")
                    for c in range(C):
                        nc.tensor.matmul(p[:], slab[:, c, :], o_T[:, c, tb(th)],
                                         start=(c == 0), stop=(c == C - 1))
                    h1t = st.tile([128, TB], F16, tag="h1t")
                    nc.sync.dma_start(h1t[:], h1_d[f, :, tb(th)])
                    nc.vector.tensor_add(h2f16[:, f, tb(th)], p[:], h1t[:])
                    sq = st.tile([128, TB], F16, tag="sq")
                    nc.scalar.activation(sq[:], h2f16[:, f, tb(th)], AF.Square)
                    if f == 0:
                        nc.vector.tensor_copy(sqaccs[th][:], sq[:])
                    else:
                        nc.vector.tensor_add(sqaccs[th][:], sqaccs[th][:], sq[:])
            for th in range(NTB):
                rms_to_srep(sqaccs[th][:], s2_rep[:, tb(th)], st, psr)

        # ---------------- P5: SwiGLU MLP ----------------
        delta16 = big.tile([128, C, T], F16, tag="X", name="delta16")
        with nc.named_scope("P5"), ExitStack() as s5:
            wsl = s5.enter_context(tc.tile_pool(name="wsl5", bufs=2))
            dslp = s5.enter_context(tc.tile_pool(name="dsl5", bufs=2))
            agp = s5.enter_context(tc.tile_pool(name="ag5", bufs=1))
            mt = s5.enter_context(tc.tile_pool(name="mt5", bufs=3))
            st = s5.enter_context(tc.tile_pool(name="st5", bufs=3))
            sqp = s5.enter_context(tc.tile_pool(name="sq5", bufs=2))
            ps = s5.enter_context(tc.tile_pool(name="ps5", bufs=2, space="PSUM"))
            psr = s5.enter_context(tc.tile_pool(name="psr5", bufs=1, space="PSUM"))
            a_gs = [agp.tile([128, JG, TB], F16, tag=f"ag{th}", name=f"a_g{th}")
                    for th in range(NTB)]
            for g in range(NG):
                for jj in range(JG):
                    j = g * JG + jj
                    gs = wsl.tile([128, C, 128], F16, tag="wgu")
                    nc.sync.dma_start(gs[:], d["wg"][j])
                    us = wsl.tile([128, C, 128], F16, tag="wgu")
                    nc.sync.dma_start(us[:], d["wu"][j])
                    for th in range(NTB):
                        psg = ps.tile([128, TB], F32, tag="g")
                        psu = ps.tile([128, TB], F32, tag="u")
                        for c in range(C):
                            nc.tensor.matmul(psg[:], gs[:, c, :], h2f16[:, c, tb(th)],
                                             start=(c == 0), stop=(c == C - 1))
                        for c in range(C):
                            nc.tensor.matmul(psu[:], us[:, c, :], h2f16[:, c, tb(th)],
                                             start=(c == 0), stop=(c == C - 1))
                        t1 = mt.tile([128, TB], F16, tag="gt")
                        nc.vector.tensor_tensor(t1[:], psg[:], s2_rep[:, tb(th)],
                                                AL.mult)
                        sg = mt.tile([128, TB], F16, tag="sg")
                        nc.scalar.activation(sg[:], t1[:], AF.Silu)
                        nc.vector.tensor_tensor(a_gs[th][:, jj, :], sg[:], psu[:],
                                                AL.mult)
                for f in range(C):
                    dsl = dslp.tile([128, JG, 128], F16, tag="wd")
                    nc.sync.dma_start(dsl[:], d["wd"][g, f])
                    for th in range(NTB):
                        pd = ps.tile([128, TB], F32, tag="dd")
                        for jj in range(JG):
                            nc.tensor.matmul(pd[:], dsl[:, jj, :], a_gs[th][:, jj, :],
                                             start=(jj == 0), stop=(jj == JG - 1))
                        if g == 0:
                            nc.vector.tensor_copy(delta16[:, f, tb(th)], pd[:])
                        else:
                            nc.vector.tensor_add(delta16[:, f, tb(th)],
                                                 delta16[:, f, tb(th)], pd[:])
            # h3 = h2 + delta*s2 (in place over h2f16), then final norm scale
            for th in range(NTB):
                sqacc = sqp.tile([128, TB], F32, tag="acc")
                for f in range(C):
                    dt = st.tile([128, TB], F16, tag="dt")
                    nc.vector.tensor_tensor(dt[:], delta16[:, f, tb(th)],
                                            s2_rep[:, tb(th)], AL.mult)
                    nc.vector.tensor_add(h2f16[:, f, tb(th)], h2f16[:, f, tb(th)],
                                         dt[:])
                    sq = st.tile([128, TB], F16, tag="sq")
                    nc.scalar.activation(sq[:], h2f16[:, f, tb(th)], AF.Square)
                    if f == 0:
                        nc.vector.tensor_copy(sqacc[:], sq[:])
                    else:
                        nc.vector.tensor_add(sqacc[:], sqacc[:], sq[:])
                rms_to_srep(sqacc[:], s3_rep[:, tb(th)], st, psr)
                for f in range(C):
                    nc.vector.tensor_tensor(h2f16[:, f, tb(th)],
                                            h2f16[:, f, tb(th)],
                                            s3_rep[:, tb(th)], AL.mult)

        # ---------------- P6: head ----------------
        with nc.named_scope("P6"), ExitStack() as s6:
            st = s6.enter_context(tc.tile_pool(name="st6", bufs=2))
            wfcp = s6.enter_context(tc.tile_pool(name="wfcp", bufs=8))
            ps = s6.enter_context(tc.tile_pool(name="ps6", bufs=1, space="PSUM"))
            h3n_r = h2f16[:].rearrange("p c (s n) -> p c n s", n=N)
            psz = ps.tile([SEQ, FC], F32, tag="z")
            for t in range(N):
                slab = wfcp.tile([128, C, FC], F16, tag="wfc")
                nc.sync.dma_start(slab[:], d["wfc"][t])
                for dc in range(C):
                    nc.tensor.matmul(psz[:], h3n_r[:, dc, t, :], slab[:, dc, :],
                                     start=(t == 0 and dc == 0),
                                     stop=(t == N - 1 and dc == C - 1))
            z1 = st.tile([SEQ, FC], F32, tag="z1")
            nc.vector.tensor_add(z1[:], psz[:], bfc_sb[:])
            zl = st.tile([SEQ, FC], F16, tag="zl")
            nc.scalar.activation(zl[:], z1[:], AF.Lrelu, alpha=0.01)
            pzt = ps.tile([FC, SEQ], F16, tag="zt")
            nc.tensor.transpose(pzt[:], zl[:], ident[0:SEQ, 0:SEQ])
            zT = st.tile([FC, SEQ], F16, tag="zT")
            nc.vector.tensor_copy(zT[:], pzt[:])
            ps2o = ps.tile([PRED, SEQ], F32, tag="o2")
            nc.tensor.matmul(ps2o[:], wout_sb[:], zT[:], start=True, stop=True)
            osb = st.tile([PRED, SEQ], F32, tag="osb")
            nc.vector.tensor_tensor(
                osb[:], ps2o[:],
                bout_sb[:, 0:1].to_broadcast((PRED, SEQ)), AL.add)
            nc.sync.dma_start(out_d[:], osb[:])

    nc.finalize()
    return nc


# ---------------- host side ----------------

def _prep_weights(inputs):
    f16 = np.float16
    anw = np.asarray(inputs["attn_norm_w"], np.float32)
    mnw = np.asarray(inputs["mlp_norm_w"], np.float32)
    fnw = np.asarray(inputs["final_norm_w"], np.float32)

    def swz(wT, nf):  # [Din, nf*128] -> [nf, 128ci, C, 128m]
        return np.ascontiguousarray(
            wT.reshape(C, 128, nf, 128).transpose(2, 1, 0, 3).astype(f16))

    w = {}
    w["w_in"] = np.zeros((128, C, 128), f16)
    w["w_in"][:P_PATCH] = (np.asarray(inputs["W_in"], np.float32).T
                           .reshape(P_PATCH, C, 128).astype(f16))
    w["w_in"][P_PATCH] = (np.asarray(inputs["b_in"], np.float32)
                          .reshape(C, 128).astype(f16))
    w["wq"] = swz((np.asarray(inputs["Wq"], np.float32) * anw[None, :]).T, H)
    w["wk"] = swz((np.asarray(inputs["Wk"], np.float32) * anw[None, :]).T, KV)
    w["wv"] = swz((np.asarray(inputs["Wv"], np.float32) * anw[None, :]).T, KV)
    w["wo"] = swz(np.asarray(inputs["Wo"], np.float32).T, C)
    w["wg"] = swz((np.asarray(inputs["Wg"], np.float32) * mnw[None, :]).T, JC)
    w["wu"] = swz((np.asarray(inputs["Wu"], np.float32) * mnw[None, :]).T, JC)
    wdT = np.asarray(inputs["Wd"], np.float32).T          # [DFF, D]
    wd5 = wdT.reshape(NG, JG, 128, C, 128)                # [g, jj, ji, f, m]
    w["wd"] = np.ascontiguousarray(wd5.transpose(0, 3, 2, 1, 4).astype(f16))
    wfcT = (np.asarray(inputs["W_fc"], np.float32).reshape(FC, N, D)
            * fnw[None, None, :]).reshape(FC, N * D).T    # [N*D, FC]
    w["wfc"] = np.ascontiguousarray(
        wfcT.reshape(N, C, 128, FC).transpose(0, 2, 1, 3).astype(f16))
    w["wout"] = np.ascontiguousarray(
        np.asarray(inputs["W_out"], np.float32).T.astype(f16))
    w["b_fc"] = np.asarray(inputs["b_fc"], np.float32).reshape(1, FC).copy()
    w["b_out"] = np.asarray(inputs["b_out"], np.float32).reshape(PRED, 1).copy()

    inv_freq = 1.0 / (THETA ** (np.arange(0, HD, 2, dtype=np.float32) / HD))
    ang = np.arange(N, dtype=np.float32)[:, None] * inv_freq[None, :]
    cos_h = np.cos(ang).T.astype(np.float32)              # [64, N]
    sin_h = np.sin(ang).T.astype(np.float32)
    cos_t = np.concatenate([cos_h, cos_h], 0)
    sin_t = np.concatenate([-sin_h, sin_h], 0)            # sign-folded
    sc = 1.0 / math.sqrt(HD)
    w["cos_q"] = np.ascontiguousarray(cos_t * sc)
    w["sin_q"] = np.ascontiguousarray(sin_t * sc)
    w["cos_k"] = np.ascontiguousarray(cos_t)
    w["sin_k"] = np.ascontiguousarray(sin_t)
    kk = np.arange(N)[:, None]
    qq = np.arange(N)[None, :]
    w["mask"] = np.ascontiguousarray((kk <= qq).astype(f16))
    return w


def _prep_patches(x):
    means = x.mean(axis=1, keepdims=True)                 # (16, 1, 7)
    stdev = np.sqrt(x.var(axis=1) + EPS)                  # (16, 7)
    xn = (x - means) / stdev[:, None, :]
    xt = xn.transpose(0, 2, 1).reshape(B * M, L)
    xp = np.concatenate([xt, np.repeat(xt[:, -1:], STRIDE, 1)], 1)
    idx = np.arange(N)[:, None] * STRIDE + np.arange(P_PATCH)[None, :]
    patches = xp[:, idx]                                  # (112, 64, 16)
    return patches, means, stdev


def _core_patch_tile(patches, core):
    pc = patches[core * SEQ:(core + 1) * SEQ]
    pt = np.zeros((128, T), np.float16)
    pt[:P_PATCH] = pc.reshape(T, P_PATCH).T.astype(np.float16)
    pt[P_PATCH] = 1.0                                     # bias row
    return pt


def make_in_maps(inputs):
    x = np.asarray(inputs["x"], np.float32)
    patches, means, stdev = _prep_patches(x)
    w = _prep_weights(inputs)
    in_maps = []
    for core in range(NCORES):
        m = dict(w)
        m["patches"] = _core_patch_tile(patches, core)
        in_maps.append(m)
    return in_maps, means, stdev


_NC_CACHE = {}


def get_nc():
    if "nc" not in _NC_CACHE:
        _NC_CACHE["nc"] = build_nc()
    return _NC_CACHE["nc"]


def kernel(**inputs) -> np.ndarray:
    in_maps, means, stdev = make_in_maps(inputs)
    nc = get_nc()
    res = bass_utils.run_bass_kernel_spmd(nc, in_maps, core_ids=list(range(NCORES)))

    out = np.zeros((B, PRED, M), np.float32)
    for core in range(NCORES):
        oc = res.results[core]["out"]                     # (96, 14)
        for sl in range(SEQ):
            s = core * SEQ + sl
            b, mi = divmod(s, M)
            out[b, :, mi] = oc[:, sl] * stdev[b, mi] + means[b, 0, mi]
    return out
